# revision 54
# baseline (speedup 1.0000x reference)
"""AmplitudeQuantumNet Trainium2 kernel (8-core data parallel).

Pipeline per core (128 samples):
  conv1(1->32,3x3)+BN+bias -> maxpool2 -> relu   [K=76 im2col matmul (4x replicated
                                                  rows to keep the PE clock warm)]
  conv2(32->64,3x3)+BN -> maxpool2 -> +bias,relu [K=96 = 32ch x 3 row-parity blocks;
                                                  dx via 3 accumulated matmuls with
                                                  free-dim-shifted rhs views]
  fc(3136->256)+tanh                             [98 K=64 matmuls, pix-major rhs]
  quantum statevector sim                        [host-built 256x256 unitary]
  probs -> Z expvals (+norm via unitarity) -> MLP
"""

import sys

sys.path.insert(0, "/opt/trn_rl_repo")

import numpy as np
import ml_dtypes

BF16 = ml_dtypes.bfloat16

N_QUBITS = 8
Q_DEPTH = 10
DIM = 256
BN_EPS = 1e-5
B = 1024
NCORES = 8
B_CORE = B // NCORES  # 128
NCH = 4
CH = B_CORE // NCH    # 32 samples per chunk
KREP = 1              # conv1 K replication (kept for experimentation)
JY = [1, 3, 0, 2]     # conv1 partition-block -> jy shift (pool pairs land so
                      # that yE/yO pool output computes on lanes 64:128)

_CACHE = {}


# ---------------------------------------------------------------- host precompute
def _quantum_unitary(q_params):
    """256x256 complex matrix of the full circuit (H layer + 10x[RX layer + diag])."""
    bits = ((np.arange(DIM)[:, None] >> (N_QUBITS - 1 - np.arange(N_QUBITS))) & 1)
    ph = np.where(np.arange(N_QUBITS) % 2 == 0, 1j, np.exp(1j * np.pi / 4))
    diag = np.prod(np.power(ph[None, :], bits), axis=1)
    cz = np.ones(DIM)
    for i, j in [(0, 1), (2, 3), (4, 5), (6, 7), (1, 2), (3, 4), (5, 6)]:
        cz = cz * ((-1.0) ** (bits[:, i] * bits[:, j]))
    diagc = (diag * cz).astype(np.complex128)

    def app(M, U, w):
        M = M.reshape((2,) * N_QUBITS + (DIM,))
        M = np.moveaxis(M, w, 0)
        M = np.tensordot(U, M, axes=(1, 0))
        M = np.moveaxis(M, 0, w)
        return M.reshape(DIM, DIM)

    M = np.eye(DIM, dtype=np.complex128)
    H = np.array([[1.0, 1.0], [1.0, -1.0]]) / np.sqrt(2.0)
    for w in range(N_QUBITS):
        M = app(M, H, w)
    qw = np.asarray(q_params, np.float64).reshape(Q_DEPTH, N_QUBITS)
    X = np.array([[0.0, 1.0], [1.0, 0.0]])
    I2 = np.eye(2)
    for layer in range(Q_DEPTH):
        for w in range(N_QUBITS):
            t = qw[layer, w]
            U = np.cos(t / 2) * I2 - 1j * np.sin(t / 2) * X
            M = app(M, U, w)
        M = diagc[:, None] * M
    zsigns = (1 - 2 * bits).astype(np.float64)  # [256, 8]
    return M, zsigns


def _host_prep(inputs):
    f32 = np.float32
    x = np.asarray(inputs["x"], f32)  # [1024,1,28,28]

    inv1 = inputs["bn1_gamma"] / np.sqrt(inputs["bn1_var"] + BN_EPS)
    w1f = np.asarray(inputs["conv1_w"], f32) * inv1[:, None, None, None]
    b1f = (inputs["conv1_b"] - inputs["bn1_mean"]) * inv1 + inputs["bn1_beta"]
    inv2 = inputs["bn2_gamma"] / np.sqrt(inputs["bn2_var"] + BN_EPS)
    w2f = np.asarray(inputs["conv2_w"], f32) * inv2[:, None, None, None]
    b2f = (inputs["conv2_b"] - inputs["bn2_mean"]) * inv2 + inputs["bn2_beta"]

    # conv1 lhsT [19, 128]: rows (r6,dxc) + bias row; cols (jyblk, c)
    W1 = np.zeros((19, 128), f32)
    for blk in range(4):
        jy = JY[blk]
        for r6 in range(6):
            dy = r6 - jy
            if 0 <= dy <= 2:
                for dxc in range(3):
                    W1[r6 * 3 + dxc, blk * 32:(blk + 1) * 32] = w1f[:, 0, dy, dxc]
        W1[18, blk * 32:(blk + 1) * 32] = b1f
    # replicate per 32-partition block (the packed xim puts chunk ci's
    # im2col rows at partitions 32ci..32ci+19)
    W1Q = np.zeros((128, 128), f32)
    for c in range(4):
        W1Q[32 * c:32 * c + 19, :] = W1
    W1Q = np.ascontiguousarray(W1Q, f32).astype(BF16)

    # x im2col [1024, 19, 7, 28]: row p=(r6,dxc): xpad[s, 4q+r6, x+dxc]; row 18 = 1
    xp = np.zeros((B, 30, 30), f32)
    xp[:, 1:29, 1:29] = x[:, 0]
    xim = np.empty((B, 19, 7, 28), f32)
    for r6 in range(6):
        for dxc in range(3):
            xim[:, r6 * 3 + dxc] = xp[:, r6:r6 + 25:4, dxc:dxc + 28]
    xim[:, 18] = 1.0
    # per-core packed-partition layout [128, 32, 7, 28]: partition 32c+r =
    # im2col row r of chunk c (rows 19..31 zero)
    xim_cores = []
    for i in range(NCORES):
        xc = xim[i * B_CORE:(i + 1) * B_CORE].transpose(1, 0, 2, 3)  # [19,128,7,28]
        xq = np.zeros((128, 32, 7, 28), f32)
        for c in range(4):
            xq[32 * c:32 * c + 19] = xc[:, 32 * c:32 * c + 32]
        xim_cores.append(np.ascontiguousarray(xq).astype(BF16))

    # conv2 lhsT [128, 2, 3, 64]: partition blocks of the shared pf tile are
    # [yE@q+1 | yO@q-1 | yE@q | yO@q]; both parities run K=128 with the
    # unused block zero-weighted.
    # even rows: dy0 -> yO@q-1 (32:64), dy1 -> yE@q (64:96), dy2 -> yO@q
    # odd rows:  dy0 -> yE@q (64:96), dy1 -> yO@q (96:128), dy2 -> yE@q+1 (0:32)
    W2P = np.zeros((128, 2, 3, 64), f32)
    for dx in range(3):
        W2P[32:64, 0, dx, :] = w2f[:, :, 0, dx].T
        W2P[64:96, 0, dx, :] = w2f[:, :, 1, dx].T
        W2P[96:128, 0, dx, :] = w2f[:, :, 2, dx].T
        W2P[64:96, 1, dx, :] = w2f[:, :, 0, dx].T
        W2P[96:128, 1, dx, :] = w2f[:, :, 1, dx].T
        W2P[0:32, 1, dx, :] = w2f[:, :, 2, dx].T
    W2P = W2P.astype(BF16)

    # fc lhsT [64, 49, 2, 128]: fc_w [256, 3136] -> [mt,m,c,pix] -> [c,pix,mt,m]
    fcw = np.asarray(inputs["fc_w"], f32).reshape(2, 128, 64, 49).transpose(2, 3, 0, 1)
    fcw = np.ascontiguousarray(fcw).astype(BF16)
    fcb = np.asarray(inputs["fc_b"], f32).reshape(2, 128).T  # [128, 2]
    fcb = np.ascontiguousarray(fcb)

    M, zsigns = _quantum_unitary(np.asarray(inputs["q_params"], np.float64))
    # lhsT tiles [k128, kb2, mt2, m128]: value M[mt*128+m, kb*128+k]
    mrt = M.real.T.reshape(2, 128, 2, 128).transpose(1, 0, 2, 3)
    mit = M.imag.T.reshape(2, 128, 2, 128).transpose(1, 0, 2, 3)
    mrt = np.ascontiguousarray(mrt).astype(f32).astype(BF16)
    mit = np.ascontiguousarray(mit).astype(f32).astype(BF16)
    zext = np.ones((DIM, 9), np.float64)
    zext[:, :8] = zsigns
    zext = np.ascontiguousarray(zext.reshape(2, 128, 9).transpose(1, 0, 2)).astype(f32).astype(BF16)

    p1t = np.ascontiguousarray(np.asarray(inputs["p1_w"], f32).T).astype(BF16)  # [8,128]
    p2t = np.ascontiguousarray(np.asarray(inputs["p2_w"], f32).T).astype(BF16)  # [128,64]
    p3t = np.ascontiguousarray(np.asarray(inputs["p3_w"], f32).T).astype(BF16)  # [64,10]

    common = {
        "w1": W1Q, "w2p": W2P, "fcw": fcw, "fcb": fcb,
        "mrt": mrt, "mit": mit, "zext": zext,
        "p1t": p1t, "p2t": p2t, "p3t": p3t,
        "cb2": np.asarray(b2f, f32).reshape(64, 1),
        "pb1": np.asarray(inputs["p1_b"], f32).reshape(128, 1),
        "pb2": np.asarray(inputs["p2_b"], f32).reshape(64, 1),
        "pb3": np.asarray(inputs["p3_b"], f32).reshape(10, 1),
    }
    in_maps = []
    for i in range(NCORES):
        m = dict(common)
        m["xim"] = xim_cores[i]
        in_maps.append(m)
    return in_maps


# ---------------------------------------------------------------- bass program
def _build_bass():
    import concourse.bacc as bacc
    import concourse.mybir as mybir
    import concourse.tile as tile

    dt = mybir.dt
    AF = mybir.ActivationFunctionType
    ALU = mybir.AluOpType

    nc = bacc.Bacc("TRN2", target_bir_lowering=False, debug=False,
                   num_devices=NCORES)
    xim = nc.dram_tensor("xim", [128, CH, 7, 28], dt.bfloat16,
                         kind="ExternalInput")
    w1 = nc.dram_tensor("w1", [128, 128], dt.bfloat16, kind="ExternalInput")
    w2p = nc.dram_tensor("w2p", [128, 2, 3, 64], dt.bfloat16, kind="ExternalInput")
    fcw = nc.dram_tensor("fcw", [64, 49, 2, 128], dt.bfloat16, kind="ExternalInput")
    fcb = nc.dram_tensor("fcb", [128, 2], dt.float32, kind="ExternalInput")
    mrt = nc.dram_tensor("mrt", [128, 2, 2, 128], dt.bfloat16, kind="ExternalInput")
    mit = nc.dram_tensor("mit", [128, 2, 2, 128], dt.bfloat16, kind="ExternalInput")
    zext = nc.dram_tensor("zext", [128, 2, 9], dt.bfloat16, kind="ExternalInput")
    p1t = nc.dram_tensor("p1t", [8, 128], dt.bfloat16, kind="ExternalInput")
    p2t = nc.dram_tensor("p2t", [128, 64], dt.bfloat16, kind="ExternalInput")
    p3t = nc.dram_tensor("p3t", [64, 10], dt.bfloat16, kind="ExternalInput")
    cb2 = nc.dram_tensor("cb2", [64, 1], dt.float32, kind="ExternalInput")
    pb1 = nc.dram_tensor("pb1", [128, 1], dt.float32, kind="ExternalInput")
    pb2 = nc.dram_tensor("pb2", [64, 1], dt.float32, kind="ExternalInput")
    pb3 = nc.dram_tensor("pb3", [10, 1], dt.float32, kind="ExternalInput")
    out = nc.dram_tensor("out", [10, B_CORE], dt.float32, kind="ExternalOutput")

    K1 = 19 * KREP

    with tile.TileContext(nc) as tc:
        with tc.tile_pool(name="singles", bufs=1) as singles:
            # xim chunk 0 goes first on the gpsimd SWDGE queue (it fans a
            # transfer across all 16 DMA engines; the sync/scalar HWDGE rings
            # run ~15 GB/s per transfer). Small weights ride the HWDGE rings
            # in parallel; big late-needed weights are emitted mid-pipeline.
            ximq = singles.tile([128, CH, 7, 28], dt.bfloat16, tag="ximq")
            w1_sb = singles.tile([128, 128], dt.bfloat16, tag="w1")
            nc.scalar.dma_start(out=w1_sb, in_=w1[:, :])
            # all four chunks' im2col in one 128-partition SWDGE transfer
            nc.gpsimd.dma_start(out=ximq, in_=xim[:, :, :, :])
            w2p_sb = singles.tile([128, 2, 3, 64], dt.bfloat16, tag="w2p")
            nc.scalar.dma_start(out=w2p_sb, in_=w2p[:, :, :, :])
            cb2_sb = singles.tile([64, 1], dt.float32, tag="cb2")
            nc.scalar.dma_start(out=cb2_sb, in_=cb2[:, :])
            fcb_sb = singles.tile([128, 2], dt.float32, tag="fcb")
            nc.scalar.dma_start(out=fcb_sb, in_=fcb[:, :])
            zext_sb = singles.tile([128, 2, 9], dt.bfloat16, tag="zext")
            nc.sync.dma_start(out=zext_sb, in_=zext[:, :, :])
            p1t_sb = singles.tile([8, 128], dt.bfloat16, tag="p1t")
            nc.sync.dma_start(out=p1t_sb, in_=p1t[:, :])
            p2t_sb = singles.tile([128, 64], dt.bfloat16, tag="p2t")
            nc.sync.dma_start(out=p2t_sb, in_=p2t[:, :])
            p3t_sb = singles.tile([64, 10], dt.bfloat16, tag="p3t")
            nc.sync.dma_start(out=p3t_sb, in_=p3t[:, :])
            pb1_sb = singles.tile([128, 1], dt.float32, tag="pb1")
            nc.sync.dma_start(out=pb1_sb, in_=pb1[:, :])
            pb2_sb = singles.tile([64, 1], dt.float32, tag="pb2")
            nc.sync.dma_start(out=pb2_sb, in_=pb2[:, :])
            pb3_sb = singles.tile([10, 1], dt.float32, tag="pb3")
            nc.sync.dma_start(out=pb3_sb, in_=pb3[:, :])
            # big tail weights: created here, loaded on the SWDGE ring
            # mid-pipeline (all DRAM reads belong on SWDGE; HWDGE rings are
            # 256B-page limited for DRAM)
            fcw_sb = singles.tile([64, 49, 2, 128], dt.bfloat16, tag="fcw")
            mrt_sb = singles.tile([128, 2, 2, 128], dt.bfloat16, tag="mrt")
            mit_sb = singles.tile([128, 2, 2, 128], dt.bfloat16, tag="mit")
            ones18 = singles.tile([1, 8], dt.bfloat16, tag="ones18")
            nc.vector.memset(ones18, 1.0)

            # fc input, pix-major so fc rhs streams contiguously [64, u, j, s]
            p2full = singles.tile([64, 7, 7, B_CORE], dt.bfloat16, tag="p2full")

            # conv2 input tiles, one per chunk. Layout [128, 8q, s, 16x']
            # partition blocks: 0:32=yE@(q+1), 32:64=yO@(q-1), 64:96=yE@q,
            # 96:128=yO@q. Both row parities run K=128 matmuls with the
            # unused block zero-weighted. q=7 row and x'=0,15 stay zero.
            pf_ts = [singles.tile([128, 8, CH, 16], dt.bfloat16, tag=f"pf{i}",
                                  name=f"pf{i}") for i in range(4)]
            for t_ in pf_ts:
                nc.vector.memset(t_[64:128, :, :, 0:1], 0.0)
                nc.vector.memset(t_[64:128, :, :, 15:16], 0.0)
                nc.vector.memset(t_[64:128, 7, :, :], 0.0)
                nc.vector.memset(t_[32:64, 0, :, :], 0.0)

            with tc.tile_pool(name="p1cp", bufs=2) as p1cp, \
                 tc.tile_pool(name="rcp", bufs=2) as rcp, \
                 tc.tile_pool(name="m1p", bufs=4) as m1p, \
                 tc.tile_pool(name="psA", bufs=1, space="PSUM") as psA:

                # one global 8-bank rotation shared by conv1 and conv2 so the
                # producer->consumer chain latency is hidden 4 pairs deep
                psg = psA.tile([128, 8, 512], dt.float32, tag="psg")
                pair_ctr = [0]

                def next_pair():
                    p = pair_ctr[0] % 4
                    pair_ctr[0] += 1
                    return 2 * p, 2 * p + 1



                def conv1_chunk(ci):
                    """16 matmuls (2 samples each) + pool-x into p1c."""
                    xim_sb = ximq[32 * ci:32 * ci + 19]
                    w1_c = w1_sb[32 * ci:32 * ci + 19, :]
                    p1c = p1cp.tile([128, CH, 7, 14], dt.bfloat16, tag="p1c")
                    for t in range(0, CH // 2, 2):
                        b0, b1 = next_pair()
                        for sh in range(2):
                            nc.tensor.matmul(
                                psg[:, b0 + sh, 0:392].rearrange(
                                    "p (s q x) -> p s q x", s=2, q=7, x=28),
                                w1_c, xim_sb[:, (t + sh) * 2:(t + sh + 1) * 2],
                                start=True, stop=True,
                                tile_position=(32 * ci, 0))
                        # pool-x over the bank pair in one reduce; relu is
                        # deferred to the pool-y stage
                        c1v = psg[:, b0:b0 + 2, 0:392].rearrange(
                            "p h (sqxp two) -> p h sqxp two", two=2)
                        nc.vector.tensor_reduce(
                            p1c[:, t * 2:(t + 2) * 2].rearrange(
                                "p s q x -> p (s q x)").rearrange(
                                "p (h r) -> p h r", h=2),
                            c1v, mybir.AxisListType.X, ALU.max)
                    return p1c

                def prep_dma(ci, p1c):
                    """kick the pool-y partition-shift copies (SWDGE ring).

                    p1c blocks are jy [1, 3, 0, 2]; rc lanes 64:96 get jy1
                    (pairs with jy0 -> yE), lanes 96:128 get jy3 (pairs with
                    jy2 -> yO)."""
                    r_c = rcp.tile([128, CH, 7, 14], dt.bfloat16, tag="r_c")
                    nc.sync.dma_start(out=r_c[64:96], in_=p1c[0:32])
                    nc.scalar.dma_start(out=r_c[96:128], in_=p1c[32:64])
                    return r_c

                def prep_build(ci, p1c, r_c):
                    """pool-y into the pf tile + the q-shift copies."""
                    pf_t = pf_ts[ci]
                    # pool-y max (vector) on lanes 64:128, then relu+layout
                    # shuffle on scalar per q; 64:96 = yE@q, 96:128 = yO@q
                    t1 = rcp.tile([128, CH, 7, 14], dt.bfloat16, tag="t1")
                    nc.vector.tensor_tensor(t1[64:128], p1c[64:128],
                                            r_c[64:128], ALU.max)
                    for q in range(7):
                        nc.scalar.activation(
                            pf_t[64:128, q, :, 1:15], t1[64:128, :, q, :],
                            AF.Relu)
                    # block1 = yO@(q-1): pf[32:64, 1:8] <- pf[96:128, 0:7]
                    nc.sync.dma_start(out=pf_t[32:64, 1:8, :, :],
                                      in_=pf_t[96:128, 0:7, :, :])
                    # block0 = yE@(q+1): pf[0:32, 0:7] <- pf[64:96, 1:8]
                    nc.scalar.dma_start(out=pf_t[0:32, 0:7, :, :],
                                        in_=pf_t[64:96, 1:8, :, :])
                    return (pf_t,)

                def conv2_chunk(ci, pf_t):
                    """conv2 + pool + bias/relu into p2full for chunk ci."""
                    for w in range(CH // 4):  # waves of 4 samples
                        s0 = w * 4
                        bE, bO = next_pair()
                        for dx in range(3):
                            nc.tensor.matmul(
                                psg[0:64, bE, 0:392].rearrange(
                                    "p (u s x) -> p u s x", u=7, s=4, x=14),
                                w2p_sb[:, 0, dx, :],
                                pf_t[:, 0:7, s0:s0 + 4, dx:dx + 14],
                                start=(dx == 0), stop=(dx == 2))
                        for dx in range(3):
                            nc.tensor.matmul(
                                psg[0:64, bO, 0:392].rearrange(
                                    "p (u s x) -> p u s x", u=7, s=4, x=14),
                                w2p_sb[:, 1, dx, :],
                                pf_t[:, 0:7, s0:s0 + 4, dx:dx + 14],
                                start=(dx == 0), stop=(dx == 2))
                        # pool-y: max(E, O); only one PSUM operand allowed, so
                        # copy the O bank to SBUF on the scalar engine first
                        ob = m1p.tile([64, 392], dt.bfloat16, tag="ob")
                        nc.scalar.activation(ob, psg[0:64, bO, 0:392], AF.Copy)
                        m1 = m1p.tile([64, 7, 4, 14], dt.bfloat16, tag="m1")
                        nc.vector.tensor_tensor(
                            m1.rearrange("p u s x -> p (u s x)"),
                            psg[0:64, bE, 0:392], ob, ALU.max)
                        # pool-x: max over adjacent x -> mp [64, 7, 4, 7]
                        mp = m1p.tile([64, 7, 4, 7], dt.bfloat16, tag="mp")
                        m1v = m1.rearrange("p u s (j two) -> p u s j two", two=2)
                        nc.vector.tensor_tensor(
                            mp, m1v[:, :, :, :, 0], m1v[:, :, :, :, 1], ALU.max)
                        # bias+relu into p2full; iterate (u, j, s) so the
                        # strided p2full writes are 4-contiguous
                        dst = p2full[:, :, :, ci * CH + s0: ci * CH + s0 + 4]
                        nc.scalar.activation(
                            dst, mp.rearrange("p u s j -> p u j s"),
                            AF.Relu, bias=cb2_sb[:, 0:1])

                # ---- pipeline: interleave conv1 / conv2 chunks ----
                p1c0 = conv1_chunk(0)
                rc0 = prep_dma(0, p1c0)
                p1c1 = conv1_chunk(1)
                rc1 = prep_dma(1, p1c1)
                nc.gpsimd.dma_start(
                    out=fcw_sb.rearrange("c p m x -> c (p m x)"),
                    in_=fcw.rearrange("c p m x -> c (p m x)"))
                nc.gpsimd.dma_start(
                    out=mrt_sb.rearrange("c a b x -> c (a b x)"),
                    in_=mrt.rearrange("c a b x -> c (a b x)"))
                nc.gpsimd.dma_start(
                    out=mit_sb.rearrange("c a b x -> c (a b x)"),
                    in_=mit.rearrange("c a b x -> c (a b x)"))
                pp0 = prep_build(0, p1c0, rc0)
                p1c2 = conv1_chunk(2)
                rc2 = prep_dma(2, p1c2)
                pp1 = prep_build(1, p1c1, rc1)
                conv2_chunk(0, *pp0)
                p1c3 = conv1_chunk(3)
                rc3 = prep_dma(3, p1c3)
                pp2 = prep_build(2, p1c2, rc2)
                conv2_chunk(1, *pp1)
                pp3 = prep_build(3, p1c3, rc3)
                conv2_chunk(2, *pp2)
                conv2_chunk(3, *pp3)

            # ---------------- dense tail ----------------
            with tc.tile_pool(name="tail", bufs=1) as tail, \
                 tc.tile_pool(name="psumT", bufs=1, space="PSUM") as psumT:
                fp = psumT.tile([128, 2, 128], dt.float32, tag="fp")
                rhsfc = p2full.rearrange("c u j s -> c (u j) s")
                for mt in range(2):
                    for pix in range(49):
                        nc.tensor.matmul(
                            fp[:, mt], fcw_sb[:, pix, mt, :], rhsfc[:, pix, :],
                            start=(pix == 0), stop=(pix == 48))
                feats = tail.tile([128, 2, 128], dt.bfloat16, tag="feats")
                for mt in range(2):
                    nc.scalar.activation(feats[:, mt], fp[:, mt], AF.Tanh,
                                         bias=fcb_sb[:, mt:mt + 1])

                sq = psumT.tile([128, 4, 128], dt.float32, tag="sq")
                srp = sq[:, 0:2]
                sip = sq[:, 2:4]
                for mt in range(2):
                    for kb in range(2):
                        nc.tensor.matmul(srp[:, mt], mrt_sb[:, kb, mt, :],
                                         feats[:, kb],
                                         start=(kb == 0), stop=(kb == 1))
                    for kb in range(2):
                        nc.tensor.matmul(sip[:, mt], mit_sb[:, kb, mt, :],
                                         feats[:, kb],
                                         start=(kb == 0), stop=(kb == 1))

                probs = tail.tile([128, 2, 128], dt.bfloat16, tag="probs")
                for mt in range(2):
                    t1 = tail.tile([128, 128], dt.float32, tag=f"sq_r{mt}")
                    nc.scalar.activation(t1, srp[:, mt], AF.Square)
                    t2s = tail.tile([128, 128], dt.float32, tag=f"sq_i{mt}")
                    nc.scalar.activation(t2s, sip[:, mt], AF.Square)
                    nc.vector.tensor_tensor(probs[:, mt], t1, t2s, ALU.add)

                qt = psumT.tile([8, 2, 128], dt.float32, tag="qt")
                qp = qt[:, 0]
                tp = qt[0:1, 1]
                for kb in range(2):
                    nc.tensor.matmul(qp, zext_sb[:, kb, 0:8], probs[:, kb],
                                     start=(kb == 0), stop=(kb == 1))
                for kb in range(2):
                    nc.tensor.matmul(tp, zext_sb[:, kb, 8:9], probs[:, kb],
                                     start=(kb == 0), stop=(kb == 1))

                recip = tail.tile([1, 128], dt.float32, tag="recip")
                nc.vector.reciprocal(recip, tp)
                recip_bf = tail.tile([1, 128], dt.bfloat16, tag="recip_bf")
                nc.vector.tensor_copy(out=recip_bf, in_=recip)
                bc = psumT.tile([8, 128], dt.float32, tag="bc")
                nc.tensor.matmul(bc, ones18, recip_bf, start=True, stop=True)
                bc_sb = tail.tile([8, 128], dt.float32, tag="bc_sb")
                nc.scalar.activation(bc_sb, bc, AF.Copy)

                qn = tail.tile([8, 128], dt.bfloat16, tag="qn")
                nc.vector.tensor_tensor(qn, qp[0:8, :], bc_sb, ALU.mult)

                zp = psumT.tile([128, 3, 128], dt.float32, tag="zp")
                z1p = zp[:, 0]
                z2p = zp[0:64, 1]
                z3p = zp[0:10, 2]
                nc.tensor.matmul(z1p, p1t_sb, qn, start=True, stop=True)
                z1 = tail.tile([128, 128], dt.bfloat16, tag="z1")
                nc.scalar.activation(z1, z1p, AF.Relu, bias=pb1_sb[:, 0:1])

                nc.tensor.matmul(z2p, p2t_sb, z1, start=True, stop=True)
                z2 = tail.tile([64, 128], dt.bfloat16, tag="z2")
                nc.scalar.activation(z2, z2p, AF.Relu, bias=pb2_sb[:, 0:1])

                nc.tensor.matmul(z3p, p3t_sb, z2, start=True, stop=True)
                osb = tail.tile([10, 128], dt.float32, tag="osb")
                nc.vector.tensor_scalar_add(osb, z3p, pb3_sb[:, 0:1])
                nc.sync.dma_start(out=out[:, :], in_=osb)

    nc.finalize()
    return nc


def _get_nc():
    if "nc" not in _CACHE:
        _CACHE["nc"] = _build_bass()
    return _CACHE["nc"]


def kernel(**inputs) -> np.ndarray:
    from concourse.bass_utils import run_bass_kernel_spmd

    in_maps = _host_prep(inputs)
    nc = _get_nc()
    res = run_bass_kernel_spmd(nc, in_maps, core_ids=list(range(NCORES)),
                               trace=bool(_CACHE.get("trace")))
    _CACHE["last_result"] = res
    outs = [r["out"].T for r in res.results]  # each [128, 10]
    return np.ascontiguousarray(np.concatenate(outs, axis=0), dtype=np.float32)


# revision 55
# speedup vs baseline: 1.0146x; 1.0146x over previous
"""AmplitudeQuantumNet Trainium2 kernel (8-core data parallel).

Pipeline per core (128 samples):
  conv1(1->32,3x3)+BN+bias -> maxpool2 -> relu   [K=76 im2col matmul (4x replicated
                                                  rows to keep the PE clock warm)]
  conv2(32->64,3x3)+BN -> maxpool2 -> +bias,relu [K=96 = 32ch x 3 row-parity blocks;
                                                  dx via 3 accumulated matmuls with
                                                  free-dim-shifted rhs views]
  fc(3136->256)+tanh                             [98 K=64 matmuls, pix-major rhs]
  quantum statevector sim                        [host-built 256x256 unitary]
  probs -> Z expvals (+norm via unitarity) -> MLP
"""

import sys

sys.path.insert(0, "/opt/trn_rl_repo")

import numpy as np
import ml_dtypes

BF16 = ml_dtypes.bfloat16

N_QUBITS = 8
Q_DEPTH = 10
DIM = 256
BN_EPS = 1e-5
B = 1024
NCORES = 8
B_CORE = B // NCORES  # 128
NCH = 4
CH = B_CORE // NCH    # 32 samples per chunk
KREP = 1              # conv1 K replication (kept for experimentation)
JY = [1, 3, 0, 2]     # conv1 partition-block -> jy shift (pool pairs land so
                      # that yE/yO pool output computes on lanes 64:128)

_CACHE = {}


# ---------------------------------------------------------------- host precompute
def _quantum_unitary(q_params):
    """256x256 complex matrix of the full circuit (H layer + 10x[RX layer + diag])."""
    bits = ((np.arange(DIM)[:, None] >> (N_QUBITS - 1 - np.arange(N_QUBITS))) & 1)
    ph = np.where(np.arange(N_QUBITS) % 2 == 0, 1j, np.exp(1j * np.pi / 4))
    diag = np.prod(np.power(ph[None, :], bits), axis=1)
    cz = np.ones(DIM)
    for i, j in [(0, 1), (2, 3), (4, 5), (6, 7), (1, 2), (3, 4), (5, 6)]:
        cz = cz * ((-1.0) ** (bits[:, i] * bits[:, j]))
    diagc = (diag * cz).astype(np.complex128)

    def app(M, U, w):
        M = M.reshape((2,) * N_QUBITS + (DIM,))
        M = np.moveaxis(M, w, 0)
        M = np.tensordot(U, M, axes=(1, 0))
        M = np.moveaxis(M, 0, w)
        return M.reshape(DIM, DIM)

    M = np.eye(DIM, dtype=np.complex128)
    H = np.array([[1.0, 1.0], [1.0, -1.0]]) / np.sqrt(2.0)
    for w in range(N_QUBITS):
        M = app(M, H, w)
    qw = np.asarray(q_params, np.float64).reshape(Q_DEPTH, N_QUBITS)
    X = np.array([[0.0, 1.0], [1.0, 0.0]])
    I2 = np.eye(2)
    for layer in range(Q_DEPTH):
        for w in range(N_QUBITS):
            t = qw[layer, w]
            U = np.cos(t / 2) * I2 - 1j * np.sin(t / 2) * X
            M = app(M, U, w)
        M = diagc[:, None] * M
    zsigns = (1 - 2 * bits).astype(np.float64)  # [256, 8]
    return M, zsigns


def _host_prep(inputs):
    f32 = np.float32
    x = np.asarray(inputs["x"], f32)  # [1024,1,28,28]

    inv1 = inputs["bn1_gamma"] / np.sqrt(inputs["bn1_var"] + BN_EPS)
    w1f = np.asarray(inputs["conv1_w"], f32) * inv1[:, None, None, None]
    b1f = (inputs["conv1_b"] - inputs["bn1_mean"]) * inv1 + inputs["bn1_beta"]
    inv2 = inputs["bn2_gamma"] / np.sqrt(inputs["bn2_var"] + BN_EPS)
    w2f = np.asarray(inputs["conv2_w"], f32) * inv2[:, None, None, None]
    b2f = (inputs["conv2_b"] - inputs["bn2_mean"]) * inv2 + inputs["bn2_beta"]

    # conv1 lhsT [19, 128]: rows (r6,dxc) + bias row; cols (jyblk, c)
    W1 = np.zeros((19, 128), f32)
    for blk in range(4):
        jy = JY[blk]
        for r6 in range(6):
            dy = r6 - jy
            if 0 <= dy <= 2:
                for dxc in range(3):
                    W1[r6 * 3 + dxc, blk * 32:(blk + 1) * 32] = w1f[:, 0, dy, dxc]
        W1[18, blk * 32:(blk + 1) * 32] = b1f
    # replicate per 32-partition block (the packed xim puts chunk ci's
    # im2col rows at partitions 32ci..32ci+19)
    W1Q = np.zeros((128, 128), f32)
    for c in range(4):
        W1Q[32 * c:32 * c + 19, :] = W1
    W1Q = np.ascontiguousarray(W1Q, f32).astype(BF16)

    # x im2col [1024, 19, 7, 28]: row p=(r6,dxc): xpad[s, 4q+r6, x+dxc]; row 18 = 1
    xp = np.zeros((B, 30, 30), f32)
    xp[:, 1:29, 1:29] = x[:, 0]
    xim = np.empty((B, 19, 7, 28), f32)
    for r6 in range(6):
        for dxc in range(3):
            xim[:, r6 * 3 + dxc] = xp[:, r6:r6 + 25:4, dxc:dxc + 28]
    xim[:, 18] = 1.0
    # per-core packed-partition layout [128, 32, 7, 28]: partition 32c+r =
    # im2col row r of chunk c (rows 19..31 zero)
    xim_cores = []
    for i in range(NCORES):
        xc = xim[i * B_CORE:(i + 1) * B_CORE].transpose(1, 0, 2, 3)  # [19,128,7,28]
        xq = np.zeros((128, 32, 7, 28), f32)
        for c in range(4):
            xq[32 * c:32 * c + 19] = xc[:, 32 * c:32 * c + 32]
        xim_cores.append(np.ascontiguousarray(xq).astype(BF16))

    # conv2 lhsT [128, 2, 3, 64]: partition blocks of the shared pf tile are
    # [yE@q+1 | yO@q-1 | yE@q | yO@q]; both parities run K=128 with the
    # unused block zero-weighted.
    # even rows: dy0 -> yO@q-1 (32:64), dy1 -> yE@q (64:96), dy2 -> yO@q
    # odd rows:  dy0 -> yE@q (64:96), dy1 -> yO@q (96:128), dy2 -> yE@q+1 (0:32)
    W2P = np.zeros((128, 2, 3, 64), f32)
    for dx in range(3):
        W2P[32:64, 0, dx, :] = w2f[:, :, 0, dx].T
        W2P[64:96, 0, dx, :] = w2f[:, :, 1, dx].T
        W2P[96:128, 0, dx, :] = w2f[:, :, 2, dx].T
        W2P[64:96, 1, dx, :] = w2f[:, :, 0, dx].T
        W2P[96:128, 1, dx, :] = w2f[:, :, 1, dx].T
        W2P[0:32, 1, dx, :] = w2f[:, :, 2, dx].T
    W2P = W2P.astype(BF16)

    # fc lhsT [64, 49, 2, 128]: fc_w [256, 3136] -> [mt,m,c,pix] -> [c,pix,mt,m]
    fcw = np.asarray(inputs["fc_w"], f32).reshape(2, 128, 64, 49).transpose(2, 3, 0, 1)
    fcw = np.ascontiguousarray(fcw).astype(BF16)
    fcb = np.asarray(inputs["fc_b"], f32).reshape(2, 128).T  # [128, 2]
    fcb = np.ascontiguousarray(fcb)

    M, zsigns = _quantum_unitary(np.asarray(inputs["q_params"], np.float64))
    # lhsT tiles [k128, kb2, mt2, m128]: value M[mt*128+m, kb*128+k]
    mrt = M.real.T.reshape(2, 128, 2, 128).transpose(1, 0, 2, 3)
    mit = M.imag.T.reshape(2, 128, 2, 128).transpose(1, 0, 2, 3)
    mrt = np.ascontiguousarray(mrt).astype(f32).astype(BF16)
    mit = np.ascontiguousarray(mit).astype(f32).astype(BF16)
    zext = np.ones((DIM, 9), np.float64)
    zext[:, :8] = zsigns
    zext = np.ascontiguousarray(zext.reshape(2, 128, 9).transpose(1, 0, 2)).astype(f32).astype(BF16)

    p1t = np.ascontiguousarray(np.asarray(inputs["p1_w"], f32).T).astype(BF16)  # [8,128]
    p2t = np.ascontiguousarray(np.asarray(inputs["p2_w"], f32).T).astype(BF16)  # [128,64]
    p3t = np.ascontiguousarray(np.asarray(inputs["p3_w"], f32).T).astype(BF16)  # [64,10]

    common = {
        "w1": W1Q, "w2p": W2P, "fcw": fcw, "fcb": fcb,
        "mrt": mrt, "mit": mit, "zext": zext,
        "p1t": p1t, "p2t": p2t, "p3t": p3t,
        "cb2": np.asarray(b2f, f32).reshape(64, 1),
        "pb1": np.asarray(inputs["p1_b"], f32).reshape(128, 1),
        "pb2": np.asarray(inputs["p2_b"], f32).reshape(64, 1),
        "pb3": np.asarray(inputs["p3_b"], f32).reshape(10, 1),
    }
    in_maps = []
    for i in range(NCORES):
        m = dict(common)
        m["xim"] = xim_cores[i]
        in_maps.append(m)
    return in_maps


# ---------------------------------------------------------------- bass program
def _build_bass():
    import concourse.bacc as bacc
    import concourse.mybir as mybir
    import concourse.tile as tile

    dt = mybir.dt
    AF = mybir.ActivationFunctionType
    ALU = mybir.AluOpType

    nc = bacc.Bacc("TRN2", target_bir_lowering=False, debug=False,
                   num_devices=NCORES)
    xim = nc.dram_tensor("xim", [128, CH, 7, 28], dt.bfloat16,
                         kind="ExternalInput")
    w1 = nc.dram_tensor("w1", [128, 128], dt.bfloat16, kind="ExternalInput")
    w2p = nc.dram_tensor("w2p", [128, 2, 3, 64], dt.bfloat16, kind="ExternalInput")
    fcw = nc.dram_tensor("fcw", [64, 49, 2, 128], dt.bfloat16, kind="ExternalInput")
    fcb = nc.dram_tensor("fcb", [128, 2], dt.float32, kind="ExternalInput")
    mrt = nc.dram_tensor("mrt", [128, 2, 2, 128], dt.bfloat16, kind="ExternalInput")
    mit = nc.dram_tensor("mit", [128, 2, 2, 128], dt.bfloat16, kind="ExternalInput")
    zext = nc.dram_tensor("zext", [128, 2, 9], dt.bfloat16, kind="ExternalInput")
    p1t = nc.dram_tensor("p1t", [8, 128], dt.bfloat16, kind="ExternalInput")
    p2t = nc.dram_tensor("p2t", [128, 64], dt.bfloat16, kind="ExternalInput")
    p3t = nc.dram_tensor("p3t", [64, 10], dt.bfloat16, kind="ExternalInput")
    cb2 = nc.dram_tensor("cb2", [64, 1], dt.float32, kind="ExternalInput")
    pb1 = nc.dram_tensor("pb1", [128, 1], dt.float32, kind="ExternalInput")
    pb2 = nc.dram_tensor("pb2", [64, 1], dt.float32, kind="ExternalInput")
    pb3 = nc.dram_tensor("pb3", [10, 1], dt.float32, kind="ExternalInput")
    out = nc.dram_tensor("out", [10, B_CORE], dt.float32, kind="ExternalOutput")

    K1 = 19 * KREP

    with tile.TileContext(nc) as tc:
        with tc.tile_pool(name="singles", bufs=1) as singles:
            # xim chunk 0 goes first on the gpsimd SWDGE queue (it fans a
            # transfer across all 16 DMA engines; the sync/scalar HWDGE rings
            # run ~15 GB/s per transfer). Small weights ride the HWDGE rings
            # in parallel; big late-needed weights are emitted mid-pipeline.
            ximq = singles.tile([128, CH, 7, 28], dt.bfloat16, tag="ximq")
            w1_sb = singles.tile([128, 128], dt.bfloat16, tag="w1")
            nc.scalar.dma_start(out=w1_sb, in_=w1[:, :])
            # all four chunks' im2col in one 128-partition SWDGE transfer
            nc.gpsimd.dma_start(out=ximq, in_=xim[:, :, :, :])
            w2p_sb = singles.tile([128, 2, 3, 64], dt.bfloat16, tag="w2p")
            nc.scalar.dma_start(out=w2p_sb, in_=w2p[:, :, :, :])
            cb2_sb = singles.tile([64, 1], dt.float32, tag="cb2")
            nc.scalar.dma_start(out=cb2_sb, in_=cb2[:, :])
            fcb_sb = singles.tile([128, 2], dt.float32, tag="fcb")
            nc.scalar.dma_start(out=fcb_sb, in_=fcb[:, :])
            zext_sb = singles.tile([128, 2, 9], dt.bfloat16, tag="zext")
            nc.sync.dma_start(out=zext_sb, in_=zext[:, :, :])
            p1t_sb = singles.tile([8, 128], dt.bfloat16, tag="p1t")
            nc.sync.dma_start(out=p1t_sb, in_=p1t[:, :])
            p2t_sb = singles.tile([128, 64], dt.bfloat16, tag="p2t")
            nc.sync.dma_start(out=p2t_sb, in_=p2t[:, :])
            p3t_sb = singles.tile([64, 10], dt.bfloat16, tag="p3t")
            nc.sync.dma_start(out=p3t_sb, in_=p3t[:, :])
            pb1_sb = singles.tile([128, 1], dt.float32, tag="pb1")
            nc.sync.dma_start(out=pb1_sb, in_=pb1[:, :])
            pb2_sb = singles.tile([64, 1], dt.float32, tag="pb2")
            nc.sync.dma_start(out=pb2_sb, in_=pb2[:, :])
            pb3_sb = singles.tile([10, 1], dt.float32, tag="pb3")
            nc.sync.dma_start(out=pb3_sb, in_=pb3[:, :])
            # big tail weights: created here, loaded on the SWDGE ring
            # mid-pipeline (all DRAM reads belong on SWDGE; HWDGE rings are
            # 256B-page limited for DRAM)
            fcw_sb = singles.tile([64, 49, 2, 128], dt.bfloat16, tag="fcw")
            mrt_sb = singles.tile([128, 2, 2, 128], dt.bfloat16, tag="mrt")
            mit_sb = singles.tile([128, 2, 2, 128], dt.bfloat16, tag="mit")
            ones18 = singles.tile([1, 8], dt.bfloat16, tag="ones18")
            nc.vector.memset(ones18, 1.0)

            # fc input, pix-major so fc rhs streams contiguously [64, u, j, s]
            p2full = singles.tile([64, 7, 7, B_CORE], dt.bfloat16, tag="p2full")

            # conv2 input tiles, one per chunk. Layout [128, 8q, s, 16x']
            # partition blocks: 0:32=yE@(q+1), 32:64=yO@(q-1), 64:96=yE@q,
            # 96:128=yO@q. Both row parities run K=128 matmuls with the
            # unused block zero-weighted. q=7 row and x'=0,15 stay zero.
            pf_ts = [singles.tile([128, 8, CH, 16], dt.bfloat16, tag=f"pf{i}",
                                  name=f"pf{i}") for i in range(4)]
            for t_ in pf_ts:
                nc.vector.memset(t_[64:128, :, :, 0:1], 0.0)
                nc.vector.memset(t_[64:128, :, :, 15:16], 0.0)
                nc.vector.memset(t_[64:128, 7, :, :], 0.0)
                nc.vector.memset(t_[32:64, 0, :, :], 0.0)

            with tc.tile_pool(name="p1cp", bufs=2) as p1cp, \
                 tc.tile_pool(name="rcp", bufs=2) as rcp, \
                 tc.tile_pool(name="m1p", bufs=4) as m1p, \
                 tc.tile_pool(name="psA", bufs=1, space="PSUM") as psA:

                # one global 8-bank rotation shared by conv1 and conv2 so the
                # producer->consumer chain latency is hidden 4 pairs deep
                psg = psA.tile([128, 8, 512], dt.float32, tag="psg")
                pair_ctr = [0]

                def next_pair():
                    p = pair_ctr[0] % 4
                    pair_ctr[0] += 1
                    return 2 * p, 2 * p + 1



                def conv1_chunk(ci):
                    """16 matmuls (2 samples each) + pool-x into p1c."""
                    xim_sb = ximq[32 * ci:32 * ci + 19]
                    w1_c = w1_sb[32 * ci:32 * ci + 19, :]
                    p1c = p1cp.tile([128, CH, 7, 14], dt.bfloat16, tag="p1c")
                    for t in range(0, CH // 2, 2):
                        b0, b1 = next_pair()
                        for sh in range(2):
                            nc.tensor.matmul(
                                psg[:, b0 + sh, 0:392].rearrange(
                                    "p (s q x) -> p s q x", s=2, q=7, x=28),
                                w1_c, xim_sb[:, (t + sh) * 2:(t + sh + 1) * 2],
                                start=True, stop=True,
                                tile_position=(32 * ci, 0))
                        # pool-x over the bank pair in one reduce; relu is
                        # deferred to the pool-y stage
                        c1v = psg[:, b0:b0 + 2, 0:392].rearrange(
                            "p h (sqxp two) -> p h sqxp two", two=2)
                        nc.vector.tensor_reduce(
                            p1c[:, t * 2:(t + 2) * 2].rearrange(
                                "p s q x -> p (s q x)").rearrange(
                                "p (h r) -> p h r", h=2),
                            c1v, mybir.AxisListType.X, ALU.max)
                    return p1c

                def prep_dma(ci, p1c):
                    """kick the pool-y partition-shift copies (SWDGE ring).

                    p1c blocks are jy [1, 3, 0, 2]; rc lanes 64:96 get jy1
                    (pairs with jy0 -> yE), lanes 96:128 get jy3 (pairs with
                    jy2 -> yO)."""
                    r_c = rcp.tile([128, CH, 7, 14], dt.bfloat16, tag="r_c")
                    nc.sync.dma_start(out=r_c[64:96], in_=p1c[0:32])
                    nc.scalar.dma_start(out=r_c[96:128], in_=p1c[32:64])
                    return r_c

                def prep_build(ci, p1c, r_c):
                    """pool-y into the pf tile + the q-shift copies."""
                    pf_t = pf_ts[ci]
                    # pool-y max (vector) on lanes 64:128, then relu+layout
                    # shuffle on scalar per q; 64:96 = yE@q, 96:128 = yO@q
                    t1 = rcp.tile([128, CH, 7, 14], dt.bfloat16, tag="t1")
                    nc.vector.tensor_tensor(t1[64:128], p1c[64:128],
                                            r_c[64:128], ALU.max)
                    for q in range(7):
                        nc.scalar.activation(
                            pf_t[64:128, q, :, 1:15], t1[64:128, :, q, :],
                            AF.Relu)
                    # block1 = yO@(q-1): pf[32:64, 1:8] <- pf[96:128, 0:7]
                    nc.sync.dma_start(out=pf_t[32:64, 1:8, :, :],
                                      in_=pf_t[96:128, 0:7, :, :])
                    # block0 = yE@(q+1): pf[0:32, 0:7] <- pf[64:96, 1:8]
                    nc.scalar.dma_start(out=pf_t[0:32, 0:7, :, :],
                                        in_=pf_t[64:96, 1:8, :, :])
                    return (pf_t,)

                def conv2_chunk(ci, pf_t):
                    """conv2 + pool + bias/relu into p2full for chunk ci."""
                    for w in range(CH // 4):  # waves of 4 samples
                        s0 = w * 4
                        bE, bO = next_pair()
                        for dx in range(3):
                            nc.tensor.matmul(
                                psg[0:64, bE, 0:392].rearrange(
                                    "p (u s x) -> p u s x", u=7, s=4, x=14),
                                w2p_sb[:, 0, dx, :],
                                pf_t[:, 0:7, s0:s0 + 4, dx:dx + 14],
                                start=(dx == 0), stop=(dx == 2))
                        for dx in range(3):
                            nc.tensor.matmul(
                                psg[0:64, bO, 0:392].rearrange(
                                    "p (u s x) -> p u s x", u=7, s=4, x=14),
                                w2p_sb[:, 1, dx, :],
                                pf_t[:, 0:7, s0:s0 + 4, dx:dx + 14],
                                start=(dx == 0), stop=(dx == 2))
                        # pool-y: max(E, O); only one PSUM operand allowed, so
                        # copy the O bank to SBUF on the scalar engine first
                        ob = m1p.tile([64, 392], dt.bfloat16, tag="ob")
                        nc.scalar.activation(ob, psg[0:64, bO, 0:392], AF.Copy)
                        m1 = m1p.tile([64, 7, 4, 14], dt.bfloat16, tag="m1")
                        nc.vector.tensor_tensor(
                            m1.rearrange("p u s x -> p (u s x)"),
                            psg[0:64, bE, 0:392], ob, ALU.max)
                        # pool-x: max over adjacent x -> mp [64, 7, 4, 7]
                        mp = m1p.tile([64, 7, 4, 7], dt.bfloat16, tag="mp")
                        m1v = m1.rearrange("p u s (j two) -> p u s j two", two=2)
                        nc.vector.tensor_tensor(
                            mp, m1v[:, :, :, :, 0], m1v[:, :, :, :, 1], ALU.max)
                        # bias+relu into p2full; iterate (u, j, s) so the
                        # strided p2full writes are 4-contiguous
                        dst = p2full[:, :, :, ci * CH + s0: ci * CH + s0 + 4]
                        nc.scalar.activation(
                            dst, mp.rearrange("p u s j -> p u j s"),
                            AF.Relu, bias=cb2_sb[:, 0:1])

                # ---- pipeline: interleave conv1 / conv2 chunks ----
                p1c0 = conv1_chunk(0)
                rc0 = prep_dma(0, p1c0)
                p1c1 = conv1_chunk(1)
                rc1 = prep_dma(1, p1c1)
                nc.gpsimd.dma_start(
                    out=fcw_sb.rearrange("c p m x -> c (p m x)"),
                    in_=fcw.rearrange("c p m x -> c (p m x)"))
                nc.gpsimd.dma_start(
                    out=mrt_sb.rearrange("c a b x -> c (a b x)"),
                    in_=mrt.rearrange("c a b x -> c (a b x)"))
                nc.gpsimd.dma_start(
                    out=mit_sb.rearrange("c a b x -> c (a b x)"),
                    in_=mit.rearrange("c a b x -> c (a b x)"))
                pp0 = prep_build(0, p1c0, rc0)
                p1c2 = conv1_chunk(2)
                rc2 = prep_dma(2, p1c2)
                pp1 = prep_build(1, p1c1, rc1)
                conv2_chunk(0, *pp0)
                p1c3 = conv1_chunk(3)
                rc3 = prep_dma(3, p1c3)
                pp2 = prep_build(2, p1c2, rc2)
                conv2_chunk(1, *pp1)
                pp3 = prep_build(3, p1c3, rc3)
                conv2_chunk(2, *pp2)
                conv2_chunk(3, *pp3)

                # ---------------- dense tail ----------------
                # reuses psg banks directly (per-bank WAR staggers against the
                # last conv2 waves instead of a whole-pool barrier)
                fp = psg[:, 0, 0:256].rearrange("p (mt x) -> p mt x", mt=2)
                rhsfc = p2full.rearrange("c u j s -> c (u j) s")
                for mt in range(2):
                    for pix in range(49):
                        nc.tensor.matmul(
                            fp[:, mt], fcw_sb[:, pix, mt, :], rhsfc[:, pix, :],
                            start=(pix == 0), stop=(pix == 48))
                feats = singles.tile([128, 2, 128], dt.bfloat16, tag="feats")
                for mt in range(2):
                    nc.scalar.activation(feats[:, mt], fp[:, mt], AF.Tanh,
                                         bias=fcb_sb[:, mt:mt + 1])

                sq = psg[:, 1, :].rearrange("p (h x) -> p h x", h=4)
                srp = sq[:, 0:2]
                sip = sq[:, 2:4]
                for mt in range(2):
                    for kb in range(2):
                        nc.tensor.matmul(srp[:, mt], mrt_sb[:, kb, mt, :],
                                         feats[:, kb],
                                         start=(kb == 0), stop=(kb == 1))
                    for kb in range(2):
                        nc.tensor.matmul(sip[:, mt], mit_sb[:, kb, mt, :],
                                         feats[:, kb],
                                         start=(kb == 0), stop=(kb == 1))

                probs = singles.tile([128, 2, 128], dt.bfloat16, tag="probs")
                for mt in range(2):
                    t1 = singles.tile([128, 128], dt.float32, tag=f"sq_r{mt}")
                    nc.scalar.activation(t1, srp[:, mt], AF.Square)
                    t2s = singles.tile([128, 128], dt.float32, tag=f"sq_i{mt}")
                    nc.scalar.activation(t2s, sip[:, mt], AF.Square)
                    nc.vector.tensor_tensor(probs[:, mt], t1, t2s, ALU.add)

                qp = psg[0:8, 2, 0:128]
                tp = psg[0:1, 2, 128:256]
                for kb in range(2):
                    nc.tensor.matmul(qp, zext_sb[:, kb, 0:8], probs[:, kb],
                                     start=(kb == 0), stop=(kb == 1))
                for kb in range(2):
                    nc.tensor.matmul(tp, zext_sb[:, kb, 8:9], probs[:, kb],
                                     start=(kb == 0), stop=(kb == 1))

                recip = singles.tile([1, 128], dt.float32, tag="recip")
                nc.vector.reciprocal(recip, tp)
                recip_bf = singles.tile([1, 128], dt.bfloat16, tag="recip_bf")
                nc.vector.tensor_copy(out=recip_bf, in_=recip)
                bc = psg[0:8, 3, 0:128]
                nc.tensor.matmul(bc, ones18, recip_bf, start=True, stop=True)
                bc_sb = singles.tile([8, 128], dt.float32, tag="bc_sb")
                nc.scalar.activation(bc_sb, bc, AF.Copy)

                qn = singles.tile([8, 128], dt.bfloat16, tag="qn")
                nc.vector.tensor_tensor(qn, qp[0:8, :], bc_sb, ALU.mult)

                z1p = psg[:, 4, 0:128]
                z2p = psg[0:64, 5, 0:128]
                z3p = psg[0:10, 6, 0:128]
                nc.tensor.matmul(z1p, p1t_sb, qn, start=True, stop=True)
                z1 = singles.tile([128, 128], dt.bfloat16, tag="z1")
                nc.scalar.activation(z1, z1p, AF.Relu, bias=pb1_sb[:, 0:1])

                nc.tensor.matmul(z2p, p2t_sb, z1, start=True, stop=True)
                z2 = singles.tile([64, 128], dt.bfloat16, tag="z2")
                nc.scalar.activation(z2, z2p, AF.Relu, bias=pb2_sb[:, 0:1])

                nc.tensor.matmul(z3p, p3t_sb, z2, start=True, stop=True)
                osb = singles.tile([10, 128], dt.float32, tag="osb")
                nc.vector.tensor_scalar_add(osb, z3p, pb3_sb[:, 0:1])
                nc.sync.dma_start(out=out[:, :], in_=osb)

    nc.finalize()
    return nc


def _get_nc():
    if "nc" not in _CACHE:
        _CACHE["nc"] = _build_bass()
    return _CACHE["nc"]


def kernel(**inputs) -> np.ndarray:
    from concourse.bass_utils import run_bass_kernel_spmd

    in_maps = _host_prep(inputs)
    nc = _get_nc()
    res = run_bass_kernel_spmd(nc, in_maps, core_ids=list(range(NCORES)),
                               trace=bool(_CACHE.get("trace")))
    _CACHE["last_result"] = res
    outs = [r["out"].T for r in res.results]  # each [128, 10]
    return np.ascontiguousarray(np.concatenate(outs, axis=0), dtype=np.float32)


# revision 57
# speedup vs baseline: 1.0371x; 1.0222x over previous
"""AmplitudeQuantumNet Trainium2 kernel (8-core data parallel).

Pipeline per core (128 samples):
  conv1(1->32,3x3)+BN+bias -> maxpool2 -> relu   [K=76 im2col matmul (4x replicated
                                                  rows to keep the PE clock warm)]
  conv2(32->64,3x3)+BN -> maxpool2 -> +bias,relu [K=96 = 32ch x 3 row-parity blocks;
                                                  dx via 3 accumulated matmuls with
                                                  free-dim-shifted rhs views]
  fc(3136->256)+tanh                             [98 K=64 matmuls, pix-major rhs]
  quantum statevector sim                        [host-built 256x256 unitary]
  probs -> Z expvals (+norm via unitarity) -> MLP
"""

import sys

sys.path.insert(0, "/opt/trn_rl_repo")

import numpy as np
import ml_dtypes

BF16 = ml_dtypes.bfloat16

N_QUBITS = 8
Q_DEPTH = 10
DIM = 256
BN_EPS = 1e-5
B = 1024
NCORES = 8
B_CORE = B // NCORES  # 128
NCH = 4
CH = B_CORE // NCH    # 32 samples per chunk
KREP = 1              # conv1 K replication (kept for experimentation)
JY = [1, 3, 0, 2]     # conv1 partition-block -> jy shift (pool pairs land so
                      # that yE/yO pool output computes on lanes 64:128)

_CACHE = {}


# ---------------------------------------------------------------- host precompute
def _quantum_unitary(q_params):
    """256x256 complex matrix of the full circuit (H layer + 10x[RX layer + diag])."""
    bits = ((np.arange(DIM)[:, None] >> (N_QUBITS - 1 - np.arange(N_QUBITS))) & 1)
    ph = np.where(np.arange(N_QUBITS) % 2 == 0, 1j, np.exp(1j * np.pi / 4))
    diag = np.prod(np.power(ph[None, :], bits), axis=1)
    cz = np.ones(DIM)
    for i, j in [(0, 1), (2, 3), (4, 5), (6, 7), (1, 2), (3, 4), (5, 6)]:
        cz = cz * ((-1.0) ** (bits[:, i] * bits[:, j]))
    diagc = (diag * cz).astype(np.complex128)

    def app(M, U, w):
        M = M.reshape((2,) * N_QUBITS + (DIM,))
        M = np.moveaxis(M, w, 0)
        M = np.tensordot(U, M, axes=(1, 0))
        M = np.moveaxis(M, 0, w)
        return M.reshape(DIM, DIM)

    M = np.eye(DIM, dtype=np.complex128)
    H = np.array([[1.0, 1.0], [1.0, -1.0]]) / np.sqrt(2.0)
    for w in range(N_QUBITS):
        M = app(M, H, w)
    qw = np.asarray(q_params, np.float64).reshape(Q_DEPTH, N_QUBITS)
    X = np.array([[0.0, 1.0], [1.0, 0.0]])
    I2 = np.eye(2)
    for layer in range(Q_DEPTH):
        for w in range(N_QUBITS):
            t = qw[layer, w]
            U = np.cos(t / 2) * I2 - 1j * np.sin(t / 2) * X
            M = app(M, U, w)
        M = diagc[:, None] * M
    zsigns = (1 - 2 * bits).astype(np.float64)  # [256, 8]
    return M, zsigns


def _host_prep(inputs):
    f32 = np.float32
    x = np.asarray(inputs["x"], f32)  # [1024,1,28,28]

    inv1 = inputs["bn1_gamma"] / np.sqrt(inputs["bn1_var"] + BN_EPS)
    w1f = np.asarray(inputs["conv1_w"], f32) * inv1[:, None, None, None]
    b1f = (inputs["conv1_b"] - inputs["bn1_mean"]) * inv1 + inputs["bn1_beta"]
    inv2 = inputs["bn2_gamma"] / np.sqrt(inputs["bn2_var"] + BN_EPS)
    w2f = np.asarray(inputs["conv2_w"], f32) * inv2[:, None, None, None]
    b2f = (inputs["conv2_b"] - inputs["bn2_mean"]) * inv2 + inputs["bn2_beta"]

    # conv1 lhsT [19, 128]: rows (r6,dxc) + bias row; cols (jyblk, c)
    W1 = np.zeros((19, 128), f32)
    for blk in range(4):
        jy = JY[blk]
        for r6 in range(6):
            dy = r6 - jy
            if 0 <= dy <= 2:
                for dxc in range(3):
                    W1[r6 * 3 + dxc, blk * 32:(blk + 1) * 32] = w1f[:, 0, dy, dxc]
        W1[18, blk * 32:(blk + 1) * 32] = b1f
    # replicate per 32-partition block (the packed xim puts chunk ci's
    # im2col rows at partitions 32ci..32ci+19)
    W1Q = np.zeros((128, 128), f32)
    for c in range(4):
        W1Q[32 * c:32 * c + 19, :] = W1
    W1Q = np.ascontiguousarray(W1Q, f32).astype(BF16)

    # x im2col [1024, 19, 7, 28]: row p=(r6,dxc): xpad[s, 4q+r6, x+dxc]; row 18 = 1
    xp = np.zeros((B, 30, 30), f32)
    xp[:, 1:29, 1:29] = x[:, 0]
    xim = np.empty((B, 19, 7, 28), f32)
    for r6 in range(6):
        for dxc in range(3):
            xim[:, r6 * 3 + dxc] = xp[:, r6:r6 + 25:4, dxc:dxc + 28]
    xim[:, 18] = 1.0
    # per-core packed-partition layout [128, 32, 7, 28]: partition 32c+r =
    # im2col row r of chunk c (rows 19..31 zero)
    xim_cores = []
    for i in range(NCORES):
        xc = xim[i * B_CORE:(i + 1) * B_CORE].transpose(1, 0, 2, 3)  # [19,128,7,28]
        xq = np.zeros((128, 32, 7, 28), f32)
        for c in range(4):
            xq[32 * c:32 * c + 19] = xc[:, 32 * c:32 * c + 32]
        xim_cores.append(np.ascontiguousarray(xq).astype(BF16))

    # conv2 lhsT [128, 2, 3, 64]: partition blocks of the shared pf tile are
    # [yE@q+1 | yO@q-1 | yE@q | yO@q]; both parities run K=128 with the
    # unused block zero-weighted.
    # even rows: dy0 -> yO@q-1 (32:64), dy1 -> yE@q (64:96), dy2 -> yO@q
    # odd rows:  dy0 -> yE@q (64:96), dy1 -> yO@q (96:128), dy2 -> yE@q+1 (0:32)
    W2P = np.zeros((128, 2, 3, 64), f32)
    for dx in range(3):
        W2P[32:64, 0, dx, :] = w2f[:, :, 0, dx].T
        W2P[64:96, 0, dx, :] = w2f[:, :, 1, dx].T
        W2P[96:128, 0, dx, :] = w2f[:, :, 2, dx].T
        W2P[64:96, 1, dx, :] = w2f[:, :, 0, dx].T
        W2P[96:128, 1, dx, :] = w2f[:, :, 1, dx].T
        W2P[0:32, 1, dx, :] = w2f[:, :, 2, dx].T
    W2P = W2P.astype(BF16)

    # fc lhsT [64, 49, 2, 128]: fc_w [256, 3136] -> [mt,m,c,pix] -> [c,pix,mt,m]
    fcw = np.asarray(inputs["fc_w"], f32).reshape(2, 128, 64, 49).transpose(2, 3, 0, 1)
    fcw = np.ascontiguousarray(fcw).astype(BF16)
    fcb = np.asarray(inputs["fc_b"], f32).reshape(2, 128).T  # [128, 2]
    fcb = np.ascontiguousarray(fcb)

    M, zsigns = _quantum_unitary(np.asarray(inputs["q_params"], np.float64))
    # lhsT tiles [k128, kb2, mt2, m128]: value M[mt*128+m, kb*128+k]
    mrt = M.real.T.reshape(2, 128, 2, 128).transpose(1, 0, 2, 3)
    mit = M.imag.T.reshape(2, 128, 2, 128).transpose(1, 0, 2, 3)
    mrt = np.ascontiguousarray(mrt).astype(f32).astype(BF16)
    mit = np.ascontiguousarray(mit).astype(f32).astype(BF16)
    zext = np.ones((DIM, 9), np.float64)
    zext[:, :8] = zsigns
    zext = np.ascontiguousarray(zext.reshape(2, 128, 9).transpose(1, 0, 2)).astype(f32).astype(BF16)

    p1t = np.ascontiguousarray(np.asarray(inputs["p1_w"], f32).T).astype(BF16)  # [8,128]
    p2t = np.ascontiguousarray(np.asarray(inputs["p2_w"], f32).T).astype(BF16)  # [128,64]
    p3t = np.ascontiguousarray(np.asarray(inputs["p3_w"], f32).T).astype(BF16)  # [64,10]

    common = {
        "w1": W1Q, "w2p": W2P, "fcw": fcw, "fcb": fcb,
        "mrt": mrt, "mit": mit, "zext": zext,
        "p1t": p1t, "p2t": p2t, "p3t": p3t,
        "cb2": np.asarray(b2f, f32).reshape(64, 1),
        "pb1": np.asarray(inputs["p1_b"], f32).reshape(128, 1),
        "pb2": np.asarray(inputs["p2_b"], f32).reshape(64, 1),
        "pb3": np.asarray(inputs["p3_b"], f32).reshape(10, 1),
    }
    in_maps = []
    for i in range(NCORES):
        m = dict(common)
        m["xim"] = xim_cores[i]
        in_maps.append(m)
    return in_maps


# ---------------------------------------------------------------- bass program
def _build_bass():
    import concourse.bacc as bacc
    import concourse.mybir as mybir
    import concourse.tile as tile

    dt = mybir.dt
    AF = mybir.ActivationFunctionType
    ALU = mybir.AluOpType

    nc = bacc.Bacc("TRN2", target_bir_lowering=False, debug=False,
                   num_devices=NCORES)
    xim = nc.dram_tensor("xim", [128, CH, 7, 28], dt.bfloat16,
                         kind="ExternalInput")
    w1 = nc.dram_tensor("w1", [128, 128], dt.bfloat16, kind="ExternalInput")
    w2p = nc.dram_tensor("w2p", [128, 2, 3, 64], dt.bfloat16, kind="ExternalInput")
    fcw = nc.dram_tensor("fcw", [64, 49, 2, 128], dt.bfloat16, kind="ExternalInput")
    fcb = nc.dram_tensor("fcb", [128, 2], dt.float32, kind="ExternalInput")
    mrt = nc.dram_tensor("mrt", [128, 2, 2, 128], dt.bfloat16, kind="ExternalInput")
    mit = nc.dram_tensor("mit", [128, 2, 2, 128], dt.bfloat16, kind="ExternalInput")
    zext = nc.dram_tensor("zext", [128, 2, 9], dt.bfloat16, kind="ExternalInput")
    p1t = nc.dram_tensor("p1t", [8, 128], dt.bfloat16, kind="ExternalInput")
    p2t = nc.dram_tensor("p2t", [128, 64], dt.bfloat16, kind="ExternalInput")
    p3t = nc.dram_tensor("p3t", [64, 10], dt.bfloat16, kind="ExternalInput")
    cb2 = nc.dram_tensor("cb2", [64, 1], dt.float32, kind="ExternalInput")
    pb1 = nc.dram_tensor("pb1", [128, 1], dt.float32, kind="ExternalInput")
    pb2 = nc.dram_tensor("pb2", [64, 1], dt.float32, kind="ExternalInput")
    pb3 = nc.dram_tensor("pb3", [10, 1], dt.float32, kind="ExternalInput")
    out = nc.dram_tensor("out", [10, B_CORE], dt.float32, kind="ExternalOutput")

    K1 = 19 * KREP

    with tile.TileContext(nc) as tc:
        with tc.tile_pool(name="singles", bufs=1) as singles:
            # xim chunk 0 goes first on the gpsimd SWDGE queue (it fans a
            # transfer across all 16 DMA engines; the sync/scalar HWDGE rings
            # run ~15 GB/s per transfer). Small weights ride the HWDGE rings
            # in parallel; big late-needed weights are emitted mid-pipeline.
            ximq = singles.tile([128, CH, 7, 28], dt.bfloat16, tag="ximq")
            w1_sb = singles.tile([128, 128], dt.bfloat16, tag="w1")
            nc.scalar.dma_start(out=w1_sb, in_=w1[:, :])
            # all four chunks' im2col in one 128-partition SWDGE transfer
            nc.gpsimd.dma_start(out=ximq, in_=xim[:, :, :, :])
            w2p_sb = singles.tile([128, 2, 3, 64], dt.bfloat16, tag="w2p")
            nc.scalar.dma_start(out=w2p_sb, in_=w2p[:, :, :, :])
            cb2_sb = singles.tile([64, 1], dt.float32, tag="cb2")
            nc.scalar.dma_start(out=cb2_sb, in_=cb2[:, :])
            fcb_sb = singles.tile([128, 2], dt.float32, tag="fcb")
            nc.scalar.dma_start(out=fcb_sb, in_=fcb[:, :])
            zext_sb = singles.tile([128, 2, 9], dt.bfloat16, tag="zext")
            nc.sync.dma_start(out=zext_sb, in_=zext[:, :, :])
            p1t_sb = singles.tile([8, 128], dt.bfloat16, tag="p1t")
            nc.sync.dma_start(out=p1t_sb, in_=p1t[:, :])
            p2t_sb = singles.tile([128, 64], dt.bfloat16, tag="p2t")
            nc.sync.dma_start(out=p2t_sb, in_=p2t[:, :])
            p3t_sb = singles.tile([64, 10], dt.bfloat16, tag="p3t")
            nc.sync.dma_start(out=p3t_sb, in_=p3t[:, :])
            pb1_sb = singles.tile([128, 1], dt.float32, tag="pb1")
            nc.sync.dma_start(out=pb1_sb, in_=pb1[:, :])
            pb2_sb = singles.tile([64, 1], dt.float32, tag="pb2")
            nc.sync.dma_start(out=pb2_sb, in_=pb2[:, :])
            pb3_sb = singles.tile([10, 1], dt.float32, tag="pb3")
            nc.sync.dma_start(out=pb3_sb, in_=pb3[:, :])
            # big tail weights: created here, loaded on the SWDGE ring
            # mid-pipeline (all DRAM reads belong on SWDGE; HWDGE rings are
            # 256B-page limited for DRAM)
            fcw_sb = singles.tile([64, 49, 2, 128], dt.bfloat16, tag="fcw")
            mrt_sb = singles.tile([128, 2, 2, 128], dt.bfloat16, tag="mrt")
            mit_sb = singles.tile([128, 2, 2, 128], dt.bfloat16, tag="mit")
            ones18 = singles.tile([1, 8], dt.bfloat16, tag="ones18")
            nc.vector.memset(ones18, 1.0)
            # scratch operands for clock-warming dummy matmuls (contents are
            # garbage; results are never read)
            wu_w = singles.tile([128, 128], dt.bfloat16, tag="wu_w")
            nc.vector.memset(wu_w, 1.0)
            wu_x = singles.tile([128, 512], dt.bfloat16, tag="wu_x")
            nc.vector.memset(wu_x, 1.0)

            # fc input, pix-major so fc rhs streams contiguously [64, u, j, s]
            p2full = singles.tile([64, 7, 7, B_CORE], dt.bfloat16, tag="p2full")

            # conv2 input tiles, one per chunk. Layout [128, 8q, s, 16x']
            # partition blocks: 0:32=yE@(q+1), 32:64=yO@(q-1), 64:96=yE@q,
            # 96:128=yO@q. Both row parities run K=128 matmuls with the
            # unused block zero-weighted. q=7 row and x'=0,15 stay zero.
            pf_ts = [singles.tile([128, 8, CH, 16], dt.bfloat16, tag=f"pf{i}",
                                  name=f"pf{i}") for i in range(4)]
            for t_ in pf_ts:
                nc.vector.memset(t_[64:128, :, :, 0:1], 0.0)
                nc.vector.memset(t_[64:128, :, :, 15:16], 0.0)
                nc.vector.memset(t_[64:128, 7, :, :], 0.0)
                nc.vector.memset(t_[32:64, 0, :, :], 0.0)

            with tc.tile_pool(name="p1cp", bufs=2) as p1cp, \
                 tc.tile_pool(name="rcp", bufs=2) as rcp, \
                 tc.tile_pool(name="m1p", bufs=4) as m1p, \
                 tc.tile_pool(name="psA", bufs=1, space="PSUM") as psA:

                # one global 8-bank rotation shared by conv1 and conv2 so the
                # producer->consumer chain latency is hidden 4 pairs deep
                psg = psA.tile([128, 8, 512], dt.float32, tag="psg")
                pair_ctr = [0]

                def next_pair():
                    p = pair_ctr[0] % 4
                    pair_ctr[0] += 1
                    return 2 * p, 2 * p + 1



                def conv1_chunk(ci):
                    """16 matmuls (2 samples each) + pool-x into p1c."""
                    xim_sb = ximq[32 * ci:32 * ci + 19]
                    w1_c = w1_sb[32 * ci:32 * ci + 19, :]
                    p1c = p1cp.tile([128, CH, 7, 14], dt.bfloat16, tag="p1c")
                    for t in range(0, CH // 2, 2):
                        b0, b1 = next_pair()
                        for sh in range(2):
                            nc.tensor.matmul(
                                psg[:, b0 + sh, 0:392].rearrange(
                                    "p (s q x) -> p s q x", s=2, q=7, x=28),
                                w1_c, xim_sb[:, (t + sh) * 2:(t + sh + 1) * 2],
                                start=True, stop=True,
                                tile_position=(32 * ci, 0))
                        # pool-x over the bank pair in one reduce; relu is
                        # deferred to the pool-y stage
                        c1v = psg[:, b0:b0 + 2, 0:392].rearrange(
                            "p h (sqxp two) -> p h sqxp two", two=2)
                        nc.vector.tensor_reduce(
                            p1c[:, t * 2:(t + 2) * 2].rearrange(
                                "p s q x -> p (s q x)").rearrange(
                                "p (h r) -> p h r", h=2),
                            c1v, mybir.AxisListType.X, ALU.max)
                    return p1c

                def prep_dma(ci, p1c):
                    """kick the pool-y partition-shift copies (SWDGE ring).

                    p1c blocks are jy [1, 3, 0, 2]; rc lanes 64:96 get jy1
                    (pairs with jy0 -> yE), lanes 96:128 get jy3 (pairs with
                    jy2 -> yO)."""
                    r_c = rcp.tile([128, CH, 7, 14], dt.bfloat16, tag="r_c")
                    nc.sync.dma_start(out=r_c[64:96], in_=p1c[0:32])
                    nc.scalar.dma_start(out=r_c[96:128], in_=p1c[32:64])
                    return r_c

                def prep_build(ci, p1c, r_c):
                    """pool-y into the pf tile + the q-shift copies."""
                    pf_t = pf_ts[ci]
                    # pool-y max (vector) on lanes 64:128, then relu+layout
                    # shuffle on scalar per q; 64:96 = yE@q, 96:128 = yO@q
                    t1 = rcp.tile([128, CH, 7, 14], dt.bfloat16, tag="t1")
                    nc.vector.tensor_tensor(t1[64:128], p1c[64:128],
                                            r_c[64:128], ALU.max)
                    for q in range(7):
                        nc.scalar.activation(
                            pf_t[64:128, q, :, 1:15], t1[64:128, :, q, :],
                            AF.Relu)
                    # block1 = yO@(q-1): pf[32:64, 1:8] <- pf[96:128, 0:7]
                    nc.sync.dma_start(out=pf_t[32:64, 1:8, :, :],
                                      in_=pf_t[96:128, 0:7, :, :])
                    # block0 = yE@(q+1): pf[0:32, 0:7] <- pf[64:96, 1:8]
                    nc.scalar.dma_start(out=pf_t[0:32, 0:7, :, :],
                                        in_=pf_t[64:96, 1:8, :, :])
                    return (pf_t,)

                def conv2_chunk(ci, pf_t):
                    """conv2 + pool + bias/relu into p2full for chunk ci."""
                    for w in range(CH // 4):  # waves of 4 samples
                        s0 = w * 4
                        bE, bO = next_pair()
                        for dx in range(3):
                            nc.tensor.matmul(
                                psg[0:64, bE, 0:392].rearrange(
                                    "p (u s x) -> p u s x", u=7, s=4, x=14),
                                w2p_sb[:, 0, dx, :],
                                pf_t[:, 0:7, s0:s0 + 4, dx:dx + 14],
                                start=(dx == 0), stop=(dx == 2))
                        for dx in range(3):
                            nc.tensor.matmul(
                                psg[0:64, bO, 0:392].rearrange(
                                    "p (u s x) -> p u s x", u=7, s=4, x=14),
                                w2p_sb[:, 1, dx, :],
                                pf_t[:, 0:7, s0:s0 + 4, dx:dx + 14],
                                start=(dx == 0), stop=(dx == 2))
                        # pool-y: max(E, O); only one PSUM operand allowed, so
                        # copy the O bank to SBUF on the scalar engine first
                        ob = m1p.tile([64, 392], dt.bfloat16, tag="ob")
                        nc.scalar.activation(ob, psg[0:64, bO, 0:392], AF.Copy)
                        m1 = m1p.tile([64, 7, 4, 14], dt.bfloat16, tag="m1")
                        nc.vector.tensor_tensor(
                            m1.rearrange("p u s x -> p (u s x)"),
                            psg[0:64, bE, 0:392], ob, ALU.max)
                        # pool-x: max over adjacent x -> mp [64, 7, 4, 7]
                        mp = m1p.tile([64, 7, 4, 7], dt.bfloat16, tag="mp")
                        m1v = m1.rearrange("p u s (j two) -> p u s j two", two=2)
                        nc.vector.tensor_tensor(
                            mp, m1v[:, :, :, :, 0], m1v[:, :, :, :, 1], ALU.max)
                        # bias+relu into p2full; iterate (u, j, s) so the
                        # strided p2full writes are 4-contiguous
                        dst = p2full[:, :, :, ci * CH + s0: ci * CH + s0 + 4]
                        nc.scalar.activation(
                            dst, mp.rearrange("p u s j -> p u j s"),
                            AF.Relu, bias=cb2_sb[:, 0:1])

                def warmup(n):
                    """dummy K=128 N=512 matmuls to trigger/hold the PE's
                    2.4GHz HAM state; outputs are discarded."""
                    for _ in range(n):
                        b0, _b1 = next_pair()
                        nc.tensor.matmul(psg[:, b0, :], wu_w, wu_x,
                                         start=True, stop=True)

                # ---- pipeline: interleave conv1 / conv2 chunks ----
                warmup(24)
                p1c0 = conv1_chunk(0)
                rc0 = prep_dma(0, p1c0)
                p1c1 = conv1_chunk(1)
                rc1 = prep_dma(1, p1c1)
                nc.gpsimd.dma_start(
                    out=fcw_sb.rearrange("c p m x -> c (p m x)"),
                    in_=fcw.rearrange("c p m x -> c (p m x)"))
                nc.gpsimd.dma_start(
                    out=mrt_sb.rearrange("c a b x -> c (a b x)"),
                    in_=mrt.rearrange("c a b x -> c (a b x)"))
                nc.gpsimd.dma_start(
                    out=mit_sb.rearrange("c a b x -> c (a b x)"),
                    in_=mit.rearrange("c a b x -> c (a b x)"))
                pp0 = prep_build(0, p1c0, rc0)
                p1c2 = conv1_chunk(2)
                rc2 = prep_dma(2, p1c2)
                pp1 = prep_build(1, p1c1, rc1)
                conv2_chunk(0, *pp0)
                p1c3 = conv1_chunk(3)
                rc3 = prep_dma(3, p1c3)
                pp2 = prep_build(2, p1c2, rc2)
                conv2_chunk(1, *pp1)
                pp3 = prep_build(3, p1c3, rc3)
                conv2_chunk(2, *pp2)
                conv2_chunk(3, *pp3)
                # keep the clock warm across the conv2->fc drain
                warmup(12)

                # ---------------- dense tail ----------------
                # reuses psg banks directly (per-bank WAR staggers against the
                # last conv2 waves instead of a whole-pool barrier)
                fp = psg[:, 0, 0:256].rearrange("p (mt x) -> p mt x", mt=2)
                rhsfc = p2full.rearrange("c u j s -> c (u j) s")
                for mt in range(2):
                    for pix in range(49):
                        nc.tensor.matmul(
                            fp[:, mt], fcw_sb[:, pix, mt, :], rhsfc[:, pix, :],
                            start=(pix == 0), stop=(pix == 48))
                feats = singles.tile([128, 2, 128], dt.bfloat16, tag="feats")
                for mt in range(2):
                    nc.scalar.activation(feats[:, mt], fp[:, mt], AF.Tanh,
                                         bias=fcb_sb[:, mt:mt + 1])

                sq = psg[:, 1, :].rearrange("p (h x) -> p h x", h=4)
                srp = sq[:, 0:2]
                sip = sq[:, 2:4]
                for mt in range(2):
                    for kb in range(2):
                        nc.tensor.matmul(srp[:, mt], mrt_sb[:, kb, mt, :],
                                         feats[:, kb],
                                         start=(kb == 0), stop=(kb == 1))
                    for kb in range(2):
                        nc.tensor.matmul(sip[:, mt], mit_sb[:, kb, mt, :],
                                         feats[:, kb],
                                         start=(kb == 0), stop=(kb == 1))

                probs = singles.tile([128, 2, 128], dt.bfloat16, tag="probs")
                for mt in range(2):
                    t1 = singles.tile([128, 128], dt.float32, tag=f"sq_r{mt}")
                    nc.scalar.activation(t1, srp[:, mt], AF.Square)
                    t2s = singles.tile([128, 128], dt.float32, tag=f"sq_i{mt}")
                    nc.scalar.activation(t2s, sip[:, mt], AF.Square)
                    nc.vector.tensor_tensor(probs[:, mt], t1, t2s, ALU.add)

                qp = psg[0:8, 2, 0:128]
                tp = psg[0:1, 2, 128:256]
                for kb in range(2):
                    nc.tensor.matmul(qp, zext_sb[:, kb, 0:8], probs[:, kb],
                                     start=(kb == 0), stop=(kb == 1))
                for kb in range(2):
                    nc.tensor.matmul(tp, zext_sb[:, kb, 8:9], probs[:, kb],
                                     start=(kb == 0), stop=(kb == 1))

                recip = singles.tile([1, 128], dt.float32, tag="recip")
                nc.vector.reciprocal(recip, tp)
                recip_bf = singles.tile([1, 128], dt.bfloat16, tag="recip_bf")
                nc.vector.tensor_copy(out=recip_bf, in_=recip)
                bc = psg[0:8, 3, 0:128]
                nc.tensor.matmul(bc, ones18, recip_bf, start=True, stop=True)
                bc_sb = singles.tile([8, 128], dt.float32, tag="bc_sb")
                nc.scalar.activation(bc_sb, bc, AF.Copy)

                qn = singles.tile([8, 128], dt.bfloat16, tag="qn")
                nc.vector.tensor_tensor(qn, qp[0:8, :], bc_sb, ALU.mult)

                z1p = psg[:, 4, 0:128]
                z2p = psg[0:64, 5, 0:128]
                z3p = psg[0:10, 6, 0:128]
                nc.tensor.matmul(z1p, p1t_sb, qn, start=True, stop=True)
                z1 = singles.tile([128, 128], dt.bfloat16, tag="z1")
                nc.scalar.activation(z1, z1p, AF.Relu, bias=pb1_sb[:, 0:1])

                nc.tensor.matmul(z2p, p2t_sb, z1, start=True, stop=True)
                z2 = singles.tile([64, 128], dt.bfloat16, tag="z2")
                nc.scalar.activation(z2, z2p, AF.Relu, bias=pb2_sb[:, 0:1])

                nc.tensor.matmul(z3p, p3t_sb, z2, start=True, stop=True)
                osb = singles.tile([10, 128], dt.float32, tag="osb")
                nc.vector.tensor_scalar_add(osb, z3p, pb3_sb[:, 0:1])
                nc.sync.dma_start(out=out[:, :], in_=osb)

    nc.finalize()
    return nc


def _get_nc():
    if "nc" not in _CACHE:
        _CACHE["nc"] = _build_bass()
    return _CACHE["nc"]


def kernel(**inputs) -> np.ndarray:
    from concourse.bass_utils import run_bass_kernel_spmd

    in_maps = _host_prep(inputs)
    nc = _get_nc()
    res = run_bass_kernel_spmd(nc, in_maps, core_ids=list(range(NCORES)),
                               trace=bool(_CACHE.get("trace")))
    _CACHE["last_result"] = res
    outs = [r["out"].T for r in res.results]  # each [128, 10]
    return np.ascontiguousarray(np.concatenate(outs, axis=0), dtype=np.float32)
